# revision 36
# baseline (speedup 1.0000x reference)
"""Multi-head attention (B=4, N=2048, C=1024, H=16) on 8 TRN2 NeuronCores.

Sharding: core = 2*b + half handles batch b, heads half*8 .. half*8+7.
Each core computes QKV for its 8 heads, full attention for them, and a
partial projection (its 512 rows of W_proj). Host sums the two partials
per batch and adds the bias.

All matmul operands are fp16; accumulation stays fp32 in PSUM. The host
pre-casts weights/x to fp16 and pre-transposes x so x^T tiles DMA in
contiguously.

On-chip layout is "transposed": Q^T/K^T [d, n] come straight out of the
QKV matmuls, scores are computed as S^T[m, n] so that exp(S^T) = P^T is
directly the moving operand of the AV matmul (V chunk stationary). exp
is shifted by a constant bias (cancels in softmax) to keep P in fp16
range. Row sums of P ride along as a 65th stationary column of ones.

The whole attention phase is one global stream of 512 "units" (pair,
n-block, m-tile, head), each a [128, 512] S^T score block. Units are
grouped into exp windows that alternate between a 3-bank and a 2-bank
PSUM tile, so the scalar engine's exp stream is fully double-buffered
(while exp reads window k, the PE writes scores into window k+1). AV
matmuls are software-pipelined one window late so the in-order PE
stream never head-of-line blocks on an exp result. Everything else
(QKV for later pairs, V, the projection, softmax normalization's
broadcast+multiply) is drained from a side queue, one unit per window,
into the PE's idle time.

The softmax reciprocal runs on the DVE but only after a DMA packs the
1024 denominators from one partition row across all 128 partitions
(the DVE's iterative divide is ~8 cycles per element per lane); the
result is DMA'd back and broadcast across partitions with two K=1
matmuls, column-tiled into one PSUM bank, so a single tensor_tensor
multiply normalizes both heads of a pair.
"""

import functools
from collections import deque
from contextlib import ExitStack

import numpy as np

import concourse.bass as bass
import concourse.tile as tile
from concourse import bacc, mybir
from concourse.bass_utils import run_bass_kernel_spmd

F32 = mybir.dt.float32
F16 = mybir.dt.float16
AF = mybir.ActivationFunctionType

B, N, C = 4, 2048, 1024
H, D = 16, 64
P = 128
NCORES = 8
HPC = 8            # heads per core
PAIRS = HPC // 2   # 4
DCORE = HPC * D    # 512 attention columns per core
SCALE = float(H) ** -0.5  # 0.25 (faithful to reference: num_heads**-0.5)
EXP_BIAS = -5.0    # exp(scale*s + bias): cancels in softmax, keeps fp16 range
NB = N // 512      # 4 n blocks
NT = N // P        # 16 m tiles of 128
CT = C // P        # 8 contraction chunks
VW = D + 1         # V columns per head incl. the ones column (row sums)
MBLK = HPC * VW    # 520 v_sb columns per m-tile

LAST_RESULT = None  # BassKernelResults of the most recent run (for test.py)


def _kernel_body(tc, out_d, xt_d, wq_d, wk_d, wv_d, wp_d):
    nc = tc.nc
    with ExitStack() as ctx:
        const = ctx.enter_context(tc.tile_pool(name="const", bufs=1))
        ones_f = const.tile([P, P], F32)
        nc.vector.memset(ones_f, 1.0)
        ones_bc = const.tile([P, 64], F16)
        nc.vector.tensor_copy(ones_bc, ones_f[:, 0:64])
        ebias = const.tile([P, 1], F32)
        nc.vector.memset(ebias, EXP_BIAS)

        # attT: pair p occupies cols [p*N, (p+1)*N); partitions = 2 heads x 64
        attT_pool = ctx.enter_context(tc.tile_pool(name="attT", bufs=1))
        attT = attT_pool.tile([P, PAIRS * N], F16)

        # QK weights first (the first K^T matmul needs them immediately).
        # The host pre-arranges every weight into its exact SBUF layout,
        # so these are single fully-contiguous DMAs.
        wqk_pool = ctx.enter_context(tc.tile_pool(name="wqk", bufs=1))
        wq_sb = wqk_pool.tile([P, PAIRS * C], F16)
        wk_sb = wqk_pool.tile([P, PAIRS * C], F16)
        nc.sync.dma_start(out=wk_sb, in_=wk_d)
        nc.sync.dma_start(out=wq_sb, in_=wq_d)

        # x^T: c-chunk j at cols [j*N, (j+1)*N) (host pre-arranged). One
        # DMA per n-block (all chunks) so the first K^T block starts early.
        xt_pool = ctx.enter_context(tc.tile_pool(name="xt", bufs=1))
        xt = xt_pool.tile([P, CT * N], F16)
        xt_v = xt.rearrange("q (cc n) -> q cc n", cc=CT)
        xtd_v = xt_d.rearrange("q (cc n) -> q cc n", cc=CT)
        for nbb in range(NB):
            nc.sync.dma_start(
                out=xt_v[:, :, nbb * 512:(nbb + 1) * 512],
                in_=xtd_v[:, :, nbb * 512:(nbb + 1) * 512])

        # V: m-tile m at cols [m*MBLK, ...); head hl at [m*MBLK + hl*VW, +D],
        # then a ones column (for row sums)
        v_pool = ctx.enter_context(tc.tile_pool(name="v", bufs=1))
        v_sb = v_pool.tile([P, NT * MBLK], F16)
        ones_cols = v_sb.rearrange("q (g k) -> q g k", k=VW)[:, :, D:VW]
        nc.vector.tensor_copy(
            ones_cols, ones_f.rearrange("q (g k) -> q g k", k=1))

        wv_pool = ctx.enter_context(tc.tile_pool(name="wv", bufs=1))
        wv_sb = wv_pool.tile([P, CT * DCORE], F16)
        nc.sync.dma_start(out=wv_sb, in_=wv_d)

        wp_pool = ctx.enter_context(tc.tile_pool(name="wp", bufs=1))
        wp_sb = wp_pool.tile([P, PAIRS * C], F16)
        nc.sync.dma_start(out=wp_sb, in_=wp_d)

        # Q^T/K^T for ALL pairs stay resident (4KB/partition each)
        qk_pool = ctx.enter_context(tc.tile_pool(name="qk", bufs=1))
        qts = [qk_pool.tile([P, N], F16, name=f"qt{i}", tag=f"qt{i}")
               for i in range(PAIRS)]
        kts = [qk_pool.tile([P, N], F16, name=f"kt{i}", tag=f"kt{i}")
               for i in range(PAIRS)]

        pt_pool = ctx.enter_context(tc.tile_pool(name="pt", bufs=3))
        den_pool = ctx.enter_context(tc.tile_pool(name="den", bufs=2))
        denp_pool = ctx.enter_context(tc.tile_pool(name="denp", bufs=2))
        rp_pool = ctx.enter_context(tc.tile_pool(name="rp", bufs=2))
        r_pool = ctx.enter_context(tc.tile_pool(name="r", bufs=2))
        tmb_pool = ctx.enter_context(tc.tile_pool(name="tmb", bufs=3))
        stage_pool = ctx.enter_context(tc.tile_pool(name="stage", bufs=3))

        # PSUM: sA 3 + sB 2 + avA/avB 2 + mm 1 = 8 banks
        ps_s = ctx.enter_context(tc.tile_pool(name="ps_s", bufs=1, space="PSUM"))
        ps_av = ctx.enter_context(tc.tile_pool(name="ps_av", bufs=1, space="PSUM"))
        ps_mm = ctx.enter_context(tc.tile_pool(name="ps_mm", bufs=1, space="PSUM"))

        def emit_qk_block(w_sb, pq, nbb, dst):
            """One n-block of Q^T or K^T for pair pq into dst[:, nbb*512...]."""
            psq = ps_mm.tile([P, 512], F32, tag="mm")
            for cc in range(CT):
                nc.tensor.matmul(
                    psq,
                    w_sb[:, pq * C + cc * P: pq * C + (cc + 1) * P],
                    xt[:, cc * N + nbb * 512: cc * N + nbb * 512 + 512],
                    start=(cc == 0), stop=(cc == CT - 1))
            nc.vector.tensor_copy(dst[:, nbb * 512:(nbb + 1) * 512], psq)

        def emit_v_chunk(pq, m):
            """V for head pair pq, m-tile m (2 heads x 64 dims)."""
            psv = ps_mm.tile([P, P], F32, tag="mm")
            for cc in range(CT):
                nc.tensor.matmul(
                    psv,
                    xt[:, cc * N + m * P: cc * N + (m + 1) * P],
                    wv_sb[:, cc * DCORE + pq * P: cc * DCORE + (pq + 1) * P],
                    start=(cc == 0), stop=(cc == CT - 1))
            base = m * MBLK + 2 * pq * VW
            nc.vector.tensor_copy(
                v_sb[:, base: base + 2 * VW].rearrange(
                    "q (h k) -> q h k", k=VW)[:, :, 0:D],
                psv.rearrange("q (h k) -> q h k", k=D))

        def emit_proj_unit(i, co, tag="mm"):
            """One [128, 512] block of this core's partial projection."""
            psp = ps_mm.tile([P, 512], F32, name="psp", tag=tag) \
                if tag == "mm" else \
                ps_s.tile([P, 512], F32, name="psp", tag=tag)
            for dc in range(PAIRS):
                nc.tensor.matmul(
                    psp,
                    attT[:, dc * N + i * P: dc * N + (i + 1) * P],
                    wp_sb[:, dc * C + co * 512: dc * C + co * 512 + 512],
                    start=(dc == 0), stop=(dc == PAIRS - 1))
            st = stage_pool.tile([P, 512], F32, tag="st")
            nc.vector.tensor_copy(st, psp)
            nc.sync.dma_start(
                out=out_d[i * P:(i + 1) * P, co * 512: co * 512 + 512],
                in_=st)

        side = deque()

        def emit_norm_front(p, nb, ava, avb):
            """DVE/DMA part of softmax normalization for (pair, nb): raw
            evictions, denominator extraction + packed reciprocal. No PE
            ops (those would head-of-line block the PE stream on the
            multi-us recip chain) — the broadcast+mul go on the side
            queue via emit_norm_back."""
            osl = slice(p * N + nb * 512, p * N + nb * 512 + 512)
            den = den_pool.tile([P, 1024], F32, tag="den")
            nc.vector.tensor_copy(den[64:65, 0:512], ava[D:VW, :])
            nc.vector.tensor_copy(den[64:65, 512:1024], avb[D:VW, :])
            nc.vector.tensor_copy(attT[0:64, osl], ava[0:64, :])
            tmb = tmb_pool.tile([64, 512], F16, tag="tmb")
            nc.vector.tensor_copy(tmb, avb[0:64, :])
            # head B's rows sit at partitions 0-63; shift to 64-127
            nc.sync.dma_start(out=attT[64:128, osl], in_=tmb)
            # r = 1/den: spread the 1024 denominators across all 128
            # partitions (DVE recip is ~8 cyc/elem PER LANE), recip, unpack
            denP = denp_pool.tile([P, 8], F32, tag="denp")
            nc.sync.dma_start(out=denP, in_=den[64:65, :])
            rP = rp_pool.tile([P, 8], F16, tag="rp")
            with nc.allow_low_precision(
                    reason="softmax recip rounding is benign"):
                nc.vector.reciprocal(rP, denP)
            r = r_pool.tile([P, 1024], F16, tag="r")
            nc.sync.dma_start(out=r[64:65, :], in_=rP)
            return r

        def emit_norm_back(p, nb, r):
            """PE+DVE tail of normalization: broadcast r across partitions
            (two K=1 matmuls col-tiled into one bank) and multiply."""
            osl = slice(p * N + nb * 512, p * N + nb * 512 + 512)
            ps_bc = ps_mm.tile([P, 512], F32, tag="mm")
            nc.tensor.matmul(
                ps_bc[0:64, :], ones_bc[64:65, :], r[64:65, 0:512],
                start=True, stop=True, tile_position=(64, 0),
                skip_group_check=True)
            nc.tensor.matmul(
                ps_bc[64:128, :], ones_bc[64:65, :], r[64:65, 512:1024],
                start=True, stop=True, tile_position=(64, 64),
                skip_group_check=True)
            with nc.allow_low_precision(
                    reason="softmax normalization rounding is benign"):
                nc.vector.tensor_mul(attT[:, osl], attT[:, osl], ps_bc)
            if p == PAIRS - 1:
                for k, i in enumerate(range(4 * nb, 4 * nb + 4)):
                    for co in range(2):
                        # the last block drains at the kernel tail where the
                        # score banks are free: double-buffer across pools
                        tag = ("sB" if (nb == NB - 1 and (k * 2 + co) % 2)
                               else "mm")
                        side.append(functools.partial(emit_proj_unit, i, co, tag))

        # ---- startup ----
        # Warm the PE's HAM clock gate with ~20 throwaway matmuls on
        # whatever attT holds (results discarded, bank overwritten later)
        # so the first real matmuls run at 2.4 GHz instead of 1.2.
        ps_warm = ps_mm.tile([P, 512], F32, tag="mm")
        for _ in range(20):
            nc.tensor.matmul(ps_warm, attT[:, 0:P], attT[:, 0:512],
                             start=True, stop=True, skip_group_check=True)
        # K^T fully and Q^T n-block 0 for pair 0 upfront; Q^T blocks 1-3
        # drain from the side queue during the first n-block's windows
        for nbb in range(NB):
            emit_qk_block(wk_sb, 0, nbb, kts[0])
        emit_qk_block(wq_sb, 0, 0, qts[0])
        for nbb in range(1, NB):
            side.append(functools.partial(emit_qk_block, wq_sb, 0, nbb, qts[0]))

        # ---- the global attention stream ----
        units = [(p, nb, m, h)
                 for p in range(PAIRS) for nb in range(NB)
                 for m in range(NT) for h in range(2)]
        pending = []        # last window's (unit, pt, slot) awaiting AV
        av_cur = {}         # (p, nb) -> (avA, avB) accumulation tiles
        v0_done = set()     # pair-0 m-tiles whose V has been emitted
        started = set()     # pairs whose side work has been queued

        def emit_av(u, pt, slot):
            p, nb, m, h = u
            if (p, nb) not in av_cur:
                av_cur[(p, nb)] = (
                    ps_av.tile([P, 512], F32, name="ava", tag="avA"),
                    ps_av.tile([P, 512], F32, name="avb", tag="avB"))
            ava, avb = av_cur[(p, nb)]
            vbase = m * MBLK + (2 * p + h) * VW
            nc.tensor.matmul(
                (ava if h == 0 else avb)[0:VW, :],
                v_sb[:, vbase: vbase + VW],
                pt[:, slot * 512:(slot + 1) * 512],
                start=(m == 0), stop=(m == NT - 1),
                skip_group_check=True)
            if m == NT - 1 and h == 1:
                r = emit_norm_front(p, nb, ava, avb)
                side.append(functools.partial(emit_norm_back, p, nb, r))
                del av_cur[(p, nb)]

        ui = 0
        toggle = 0
        wi = 0
        while ui < len(units):
            wsize = 3 if toggle == 0 else 2
            wunits = units[ui:ui + wsize]
            for (p, nb, m, h) in wunits:
                if p not in started:
                    started.add(p)
                    if p < PAIRS - 1:
                        for nbb in range(NB):
                            side.append(functools.partial(
                                emit_qk_block, wk_sb, p + 1, nbb, kts[p + 1]))
                        for nbb in range(NB):
                            side.append(functools.partial(
                                emit_qk_block, wq_sb, p + 1, nbb, qts[p + 1]))
                        for m2 in range(NT):
                            side.append(functools.partial(
                                emit_v_chunk, p + 1, m2))
                if p == 0 and nb == 0 and m not in v0_done:
                    v0_done.add(m)
                    emit_v_chunk(0, m)
            pss = ps_s.tile([P, 512 * wsize], F32,
                            tag=("sA" if toggle == 0 else "sB"))
            for i, (p, nb, m, h) in enumerate(wunits):
                nsl = slice(nb * 512, nb * 512 + 512)
                nc.tensor.matmul(
                    pss[:, i * 512:(i + 1) * 512],
                    kts[p][h * 64:(h + 1) * 64, m * P:(m + 1) * P],
                    qts[p][h * 64:(h + 1) * 64, nsl],
                    start=True, stop=True)
            pt = pt_pool.tile([P, 512 * wsize], F16,
                              tag=("ptA" if toggle == 0 else "ptB"))
            nc.scalar.activation(pt, pss, AF.Exp, scale=SCALE, bias=ebias)
            # AV of the PREVIOUS window (software pipeline: keeps ready PE
            # work queued while this window's exp runs)
            for u, ptt, slot in pending:
                emit_av(u, ptt, slot)
            pending = [(u, pt, i) for i, u in enumerate(wunits)]
            in_p0nb0 = wunits[0][0] == 0 and wunits[0][1] == 0
            if side and (wi >= 8 or not in_p0nb0):
                side.popleft()()
            ui += wsize
            toggle ^= 1
            wi += 1
        for u, ptt, slot in pending:
            emit_av(u, ptt, slot)
        while side:
            side.popleft()()


@functools.lru_cache(maxsize=1)
def build_nc():
    nc = bacc.Bacc("TRN2", target_bir_lowering=False, debug=False)
    # all inputs are pre-arranged by the host into their SBUF layouts
    xt_d = nc.dram_tensor("xt_local", [P, CT * N], F16, kind="ExternalInput").ap()
    wq_d = nc.dram_tensor("wq", [P, PAIRS * C], F16, kind="ExternalInput").ap()
    wk_d = nc.dram_tensor("wk", [P, PAIRS * C], F16, kind="ExternalInput").ap()
    wv_d = nc.dram_tensor("wv", [P, CT * DCORE], F16, kind="ExternalInput").ap()
    wp_d = nc.dram_tensor("wp", [P, PAIRS * C], F16, kind="ExternalInput").ap()
    out_d = nc.dram_tensor("out_partial", [N, C], F32, kind="ExternalOutput").ap()
    with tile.TileContext(nc) as tc:
        _kernel_body(tc, out_d, xt_d, wq_d, wk_d, wv_d, wp_d)
    nc.compile()
    return nc


def make_in_maps(x, W_qkv, W_proj):
    def qk_layout(w):  # [C, DCORE] -> [P, (pq cc k)]
        return np.ascontiguousarray(
            w.reshape(CT, P, PAIRS, P).transpose(1, 2, 0, 3).reshape(P, PAIRS * C)
            .astype(np.float16))

    def row_layout(w):  # [C or DCORE, F] -> [P, (chunk f)]
        r = w.shape[0] // P
        return np.ascontiguousarray(
            w.reshape(r, P, w.shape[1]).transpose(1, 0, 2).reshape(P, -1)
            .astype(np.float16))

    in_maps = []
    for core in range(NCORES):
        b, half = core // 2, core % 2
        h0 = half * HPC
        in_maps.append({
            "xt_local": row_layout(x[b].T),
            "wq": qk_layout(W_qkv[:, 0 * C + h0 * D: 0 * C + h0 * D + DCORE]),
            "wk": qk_layout(W_qkv[:, 1 * C + h0 * D: 1 * C + h0 * D + DCORE]),
            "wv": row_layout(W_qkv[:, 2 * C + h0 * D: 2 * C + h0 * D + DCORE]),
            "wp": row_layout(W_proj[h0 * D: h0 * D + DCORE, :]),
        })
    return in_maps


def kernel(x, W_qkv, W_proj, b_proj, trace=False):
    x = np.asarray(x, dtype=np.float32)
    W_qkv = np.asarray(W_qkv, dtype=np.float32)
    W_proj = np.asarray(W_proj, dtype=np.float32)
    b_proj = np.asarray(b_proj, dtype=np.float32)

    nc = build_nc()
    in_maps = make_in_maps(x, W_qkv, W_proj)

    global LAST_RESULT
    res = run_bass_kernel_spmd(nc, in_maps, list(range(NCORES)), trace=trace)
    LAST_RESULT = res

    out = np.empty((B, N, C), dtype=np.float32)
    for b in range(B):
        out[b] = (res.results[2 * b]["out_partial"]
                  + res.results[2 * b + 1]["out_partial"]
                  + b_proj[None, :])
    return out


# revision 37
# speedup vs baseline: 1.0918x; 1.0918x over previous
"""Multi-head attention (B=4, N=2048, C=1024, H=16) on 8 TRN2 NeuronCores.

Sharding: core = 2*b + half handles batch b, heads half*8 .. half*8+7.
Each core computes QKV for its 8 heads, full attention for them, and a
partial projection (its 512 rows of W_proj). Host sums the two partials
per batch and adds the bias.

All matmul operands are fp16; accumulation stays fp32 in PSUM. The host
pre-casts weights/x to fp16 and pre-transposes x so x^T tiles DMA in
contiguously.

On-chip layout is "transposed": Q^T/K^T [d, n] come straight out of the
QKV matmuls, scores are computed as S^T[m, n] so that exp(S^T) = P^T is
directly the moving operand of the AV matmul (V chunk stationary). exp
is shifted by a constant bias (cancels in softmax) to keep P in fp16
range. Row sums of P ride along as a 65th stationary column of ones.

The whole attention phase is one global stream of 512 "units" (pair,
n-block, m-tile, head), each a [128, 512] S^T score block. Units are
grouped into exp windows that alternate between a 3-bank and a 2-bank
PSUM tile, so the scalar engine's exp stream is fully double-buffered
(while exp reads window k, the PE writes scores into window k+1). AV
matmuls are software-pipelined one window late so the in-order PE
stream never head-of-line blocks on an exp result. Everything else
(QKV for later pairs, V, the projection, softmax normalization's
broadcast+multiply) is drained from a side queue, one unit per window,
into the PE's idle time.

The softmax reciprocal runs on the DVE but only after a DMA packs the
1024 denominators from one partition row across all 128 partitions
(the DVE's iterative divide is ~8 cycles per element per lane); the
result is DMA'd back and broadcast across partitions with two K=1
matmuls, column-tiled into one PSUM bank, so a single tensor_tensor
multiply normalizes both heads of a pair.
"""

import functools
from collections import deque
from contextlib import ExitStack

import numpy as np

import concourse.bass as bass
import concourse.tile as tile
from concourse import bacc, mybir
from concourse.bass_utils import run_bass_kernel_spmd

F32 = mybir.dt.float32
F16 = mybir.dt.float16
AF = mybir.ActivationFunctionType

B, N, C = 4, 2048, 1024
H, D = 16, 64
P = 128
NCORES = 8
HPC = 8            # heads per core
PAIRS = HPC // 2   # 4
DCORE = HPC * D    # 512 attention columns per core
SCALE = float(H) ** -0.5  # 0.25 (faithful to reference: num_heads**-0.5)
EXP_BIAS = -5.0    # exp(scale*s + bias): cancels in softmax, keeps fp16 range
NB = N // 512      # 4 n blocks
NT = N // P        # 16 m tiles of 128
CT = C // P        # 8 contraction chunks
VW = D + 1         # V columns per head incl. the ones column (row sums)
MBLK = HPC * VW    # 520 v_sb columns per m-tile

LAST_RESULT = None  # BassKernelResults of the most recent run (for test.py)


def _kernel_body(tc, out_d, xt_d, wq_d, wk_d, wv_d, wp_d):
    nc = tc.nc
    with ExitStack() as ctx:
        const = ctx.enter_context(tc.tile_pool(name="const", bufs=1))
        ones_f = const.tile([P, P], F32)
        nc.vector.memset(ones_f, 1.0)
        ones_bc = const.tile([P, 64], F16)
        nc.vector.tensor_copy(ones_bc, ones_f[:, 0:64])
        ebias = const.tile([P, 1], F32)
        nc.vector.memset(ebias, EXP_BIAS)

        # attT: pair p occupies cols [p*N, (p+1)*N); partitions = 2 heads x 64
        attT_pool = ctx.enter_context(tc.tile_pool(name="attT", bufs=1))
        attT = attT_pool.tile([P, PAIRS * N], F16)

        # x^T: c-chunk j at cols [j*N, (j+1)*N). DMA'd in (chunk, n-block)
        # pieces so the first K^T block can start ~3us in.
        xt_pool = ctx.enter_context(tc.tile_pool(name="xt", bufs=1))
        xt = xt_pool.tile([P, CT * N], F16)
        for nbb in range(NB):
            for j in range(CT):
                nc.sync.dma_start(
                    out=xt[:, j * N + nbb * 512: j * N + nbb * 512 + 512],
                    in_=xt_d[j * P:(j + 1) * P, nbb * 512: nbb * 512 + 512])

        # V: m-tile m at cols [m*MBLK, ...); head hl at [m*MBLK + hl*VW, +D],
        # then a ones column (for row sums)
        v_pool = ctx.enter_context(tc.tile_pool(name="v", bufs=1))
        v_sb = v_pool.tile([P, NT * MBLK], F16)
        ones_cols = v_sb.rearrange("q (g k) -> q g k", k=VW)[:, :, D:VW]
        nc.vector.tensor_copy(
            ones_cols, ones_f.rearrange("q (g k) -> q g k", k=1))

        wv_pool = ctx.enter_context(tc.tile_pool(name="wv", bufs=1))
        wv_sb = wv_pool.tile([P, CT * DCORE], F16)
        for cc in range(CT):
            nc.sync.dma_start(out=wv_sb[:, cc * DCORE:(cc + 1) * DCORE],
                              in_=wv_d[cc * P:(cc + 1) * P, :])

        wp_pool = ctx.enter_context(tc.tile_pool(name="wp", bufs=1))
        wp_sb = wp_pool.tile([P, PAIRS * C], F16)
        for dc in range(PAIRS):
            nc.sync.dma_start(out=wp_sb[:, dc * C:(dc + 1) * C],
                              in_=wp_d[dc * P:(dc + 1) * P, :])

        # QK weights, all pairs: pair p at cols [p*C, (p+1)*C), chunk cc
        # within that at [cc*P, (cc+1)*P)
        wqk_pool = ctx.enter_context(tc.tile_pool(name="wqk", bufs=1))
        wq_sb = wqk_pool.tile([P, PAIRS * C], F16)
        wk_sb = wqk_pool.tile([P, PAIRS * C], F16)
        for pq in range(PAIRS):
            for w_d, w_sb in ((wq_d, wq_sb), (wk_d, wk_sb)):
                nc.sync.dma_start(
                    out=w_sb[:, pq * C:(pq + 1) * C].rearrange(
                        "q (cc f) -> q cc f", cc=CT),
                    in_=w_d[:, pq * P:(pq + 1) * P].rearrange(
                        "(cc q) f -> q cc f", q=P))

        # Q^T/K^T for ALL pairs stay resident (4KB/partition each)
        qk_pool = ctx.enter_context(tc.tile_pool(name="qk", bufs=1))
        qts = [qk_pool.tile([P, N], F16, name=f"qt{i}", tag=f"qt{i}")
               for i in range(PAIRS)]
        kts = [qk_pool.tile([P, N], F16, name=f"kt{i}", tag=f"kt{i}")
               for i in range(PAIRS)]

        pt_pool = ctx.enter_context(tc.tile_pool(name="pt", bufs=3))
        den_pool = ctx.enter_context(tc.tile_pool(name="den", bufs=2))
        denp_pool = ctx.enter_context(tc.tile_pool(name="denp", bufs=2))
        rp_pool = ctx.enter_context(tc.tile_pool(name="rp", bufs=2))
        r_pool = ctx.enter_context(tc.tile_pool(name="r", bufs=2))
        tmb_pool = ctx.enter_context(tc.tile_pool(name="tmb", bufs=3))
        stage_pool = ctx.enter_context(tc.tile_pool(name="stage", bufs=3))

        # PSUM: sA 3 + sB 2 + avA/avB 2 + mm 1 = 8 banks
        ps_s = ctx.enter_context(tc.tile_pool(name="ps_s", bufs=1, space="PSUM"))
        ps_av = ctx.enter_context(tc.tile_pool(name="ps_av", bufs=1, space="PSUM"))
        ps_mm = ctx.enter_context(tc.tile_pool(name="ps_mm", bufs=1, space="PSUM"))

        def emit_qk_block(w_sb, pq, nbb, dst):
            """One n-block of Q^T or K^T for pair pq into dst[:, nbb*512...]."""
            psq = ps_mm.tile([P, 512], F32, tag="mm")
            for cc in range(CT):
                nc.tensor.matmul(
                    psq,
                    w_sb[:, pq * C + cc * P: pq * C + (cc + 1) * P],
                    xt[:, cc * N + nbb * 512: cc * N + nbb * 512 + 512],
                    start=(cc == 0), stop=(cc == CT - 1))
            nc.vector.tensor_copy(dst[:, nbb * 512:(nbb + 1) * 512], psq)

        def emit_v_chunk(pq, m):
            """V for head pair pq, m-tile m (2 heads x 64 dims)."""
            psv = ps_mm.tile([P, P], F32, tag="mm")
            for cc in range(CT):
                nc.tensor.matmul(
                    psv,
                    xt[:, cc * N + m * P: cc * N + (m + 1) * P],
                    wv_sb[:, cc * DCORE + pq * P: cc * DCORE + (pq + 1) * P],
                    start=(cc == 0), stop=(cc == CT - 1))
            base = m * MBLK + 2 * pq * VW
            nc.vector.tensor_copy(
                v_sb[:, base: base + 2 * VW].rearrange(
                    "q (h k) -> q h k", k=VW)[:, :, 0:D],
                psv.rearrange("q (h k) -> q h k", k=D))

        def emit_proj_unit(i, co):
            """One [128, 512] block of this core's partial projection."""
            psp = ps_mm.tile([P, 512], F32, tag="mm")
            for dc in range(PAIRS):
                nc.tensor.matmul(
                    psp,
                    attT[:, dc * N + i * P: dc * N + (i + 1) * P],
                    wp_sb[:, dc * C + co * 512: dc * C + co * 512 + 512],
                    start=(dc == 0), stop=(dc == PAIRS - 1))
            st = stage_pool.tile([P, 512], F32, tag="st")
            nc.vector.tensor_copy(st, psp)
            nc.sync.dma_start(
                out=out_d[i * P:(i + 1) * P, co * 512: co * 512 + 512],
                in_=st)

        side = deque()

        def emit_norm_front(p, nb, ava, avb):
            """DVE/DMA part of softmax normalization for (pair, nb): raw
            evictions, denominator extraction + packed reciprocal. No PE
            ops (those would head-of-line block the PE stream on the
            multi-us recip chain) — the broadcast+mul go on the side
            queue via emit_norm_back."""
            osl = slice(p * N + nb * 512, p * N + nb * 512 + 512)
            den = den_pool.tile([P, 1024], F32, tag="den")
            nc.vector.tensor_copy(den[64:65, 0:512], ava[D:VW, :])
            nc.vector.tensor_copy(den[64:65, 512:1024], avb[D:VW, :])
            nc.vector.tensor_copy(attT[0:64, osl], ava[0:64, :])
            tmb = tmb_pool.tile([64, 512], F16, tag="tmb")
            nc.vector.tensor_copy(tmb, avb[0:64, :])
            # head B's rows sit at partitions 0-63; shift to 64-127
            nc.sync.dma_start(out=attT[64:128, osl], in_=tmb)
            # r = 1/den: spread the 1024 denominators across all 128
            # partitions (DVE recip is ~8 cyc/elem PER LANE), recip, unpack
            denP = denp_pool.tile([P, 8], F32, tag="denp")
            nc.sync.dma_start(out=denP, in_=den[64:65, :])
            rP = rp_pool.tile([P, 8], F16, tag="rp")
            with nc.allow_low_precision(
                    reason="softmax recip rounding is benign"):
                nc.vector.reciprocal(rP, denP)
            r = r_pool.tile([P, 1024], F16, tag="r")
            nc.sync.dma_start(out=r[64:65, :], in_=rP)
            return r

        def emit_norm_back(p, nb, r):
            """PE+DVE tail of normalization: broadcast r across partitions
            (two K=1 matmuls col-tiled into one bank) and multiply."""
            osl = slice(p * N + nb * 512, p * N + nb * 512 + 512)
            ps_bc = ps_mm.tile([P, 512], F32, tag="mm")
            nc.tensor.matmul(
                ps_bc[0:64, :], ones_bc[64:65, :], r[64:65, 0:512],
                start=True, stop=True, tile_position=(64, 0),
                skip_group_check=True)
            nc.tensor.matmul(
                ps_bc[64:128, :], ones_bc[64:65, :], r[64:65, 512:1024],
                start=True, stop=True, tile_position=(64, 64),
                skip_group_check=True)
            with nc.allow_low_precision(
                    reason="softmax normalization rounding is benign"):
                nc.vector.tensor_mul(attT[:, osl], attT[:, osl], ps_bc)
            if p == PAIRS - 1:
                for i in range(4 * nb, 4 * nb + 4):
                    for co in range(2):
                        side.append(functools.partial(emit_proj_unit, i, co))

        # ---- startup: K^T fully and Q^T n-block 0 for pair 0 ----
        for nbb in range(NB):
            emit_qk_block(wk_sb, 0, nbb, kts[0])
        emit_qk_block(wq_sb, 0, 0, qts[0])
        for nbb in range(1, NB):
            side.append(functools.partial(emit_qk_block, wq_sb, 0, nbb, qts[0]))

        # ---- the global attention stream ----
        units = [(p, nb, m, h)
                 for p in range(PAIRS) for nb in range(NB)
                 for m in range(NT) for h in range(2)]
        pending = []        # last window's (unit, pt, slot) awaiting AV
        av_cur = {}         # (p, nb) -> (avA, avB) accumulation tiles
        v0_done = set()     # pair-0 m-tiles whose V has been emitted
        started = set()     # pairs whose side work has been queued

        def emit_av(u, pt, slot):
            p, nb, m, h = u
            if (p, nb) not in av_cur:
                av_cur[(p, nb)] = (
                    ps_av.tile([P, 512], F32, name="ava", tag="avA"),
                    ps_av.tile([P, 512], F32, name="avb", tag="avB"))
            ava, avb = av_cur[(p, nb)]
            vbase = m * MBLK + (2 * p + h) * VW
            nc.tensor.matmul(
                (ava if h == 0 else avb)[0:VW, :],
                v_sb[:, vbase: vbase + VW],
                pt[:, slot * 512:(slot + 1) * 512],
                start=(m == 0), stop=(m == NT - 1),
                skip_group_check=True)
            if m == NT - 1 and h == 1:
                r = emit_norm_front(p, nb, ava, avb)
                side.append(functools.partial(emit_norm_back, p, nb, r))
                del av_cur[(p, nb)]

        ui = 0
        toggle = 0
        wi = 0
        while ui < len(units):
            wsize = 3 if toggle == 0 else 2
            wunits = units[ui:ui + wsize]
            for (p, nb, m, h) in wunits:
                if p not in started:
                    started.add(p)
                    if p < PAIRS - 1:
                        for nbb in range(NB):
                            side.append(functools.partial(
                                emit_qk_block, wk_sb, p + 1, nbb, kts[p + 1]))
                        for nbb in range(NB):
                            side.append(functools.partial(
                                emit_qk_block, wq_sb, p + 1, nbb, qts[p + 1]))
                        for m2 in range(NT):
                            side.append(functools.partial(
                                emit_v_chunk, p + 1, m2))
                if p == 0 and nb == 0 and m not in v0_done:
                    v0_done.add(m)
                    emit_v_chunk(0, m)
            pss = ps_s.tile([P, 512 * wsize], F32,
                            tag=("sA" if toggle == 0 else "sB"))
            for i, (p, nb, m, h) in enumerate(wunits):
                nsl = slice(nb * 512, nb * 512 + 512)
                nc.tensor.matmul(
                    pss[:, i * 512:(i + 1) * 512],
                    kts[p][h * 64:(h + 1) * 64, m * P:(m + 1) * P],
                    qts[p][h * 64:(h + 1) * 64, nsl],
                    start=True, stop=True)
            pt = pt_pool.tile([P, 512 * wsize], F16,
                              tag=("ptA" if toggle == 0 else "ptB"))
            nc.scalar.activation(pt, pss, AF.Exp, scale=SCALE, bias=ebias)
            # AV of the PREVIOUS window (software pipeline: keeps ready PE
            # work queued while this window's exp runs)
            for u, ptt, slot in pending:
                emit_av(u, ptt, slot)
            pending = [(u, pt, i) for i, u in enumerate(wunits)]
            in_p0nb0 = wunits[0][0] == 0 and wunits[0][1] == 0
            if side and (wi >= 8 or not in_p0nb0):
                side.popleft()()
            ui += wsize
            toggle ^= 1
            wi += 1
        for u, ptt, slot in pending:
            emit_av(u, ptt, slot)
        while side:
            side.popleft()()


@functools.lru_cache(maxsize=1)
def build_nc():
    nc = bacc.Bacc("TRN2", target_bir_lowering=False, debug=False)
    xt_d = nc.dram_tensor("xt_local", [C, N], F16, kind="ExternalInput").ap()
    wq_d = nc.dram_tensor("wq", [C, DCORE], F16, kind="ExternalInput").ap()
    wk_d = nc.dram_tensor("wk", [C, DCORE], F16, kind="ExternalInput").ap()
    wv_d = nc.dram_tensor("wv", [C, DCORE], F16, kind="ExternalInput").ap()
    wp_d = nc.dram_tensor("wp", [DCORE, C], F16, kind="ExternalInput").ap()
    out_d = nc.dram_tensor("out_partial", [N, C], F32, kind="ExternalOutput").ap()
    with tile.TileContext(nc) as tc:
        _kernel_body(tc, out_d, xt_d, wq_d, wk_d, wv_d, wp_d)
    nc.compile()
    return nc


def make_in_maps(x, W_qkv, W_proj):
    in_maps = []
    for core in range(NCORES):
        b, half = core // 2, core % 2
        h0 = half * HPC
        in_maps.append({
            "xt_local": np.ascontiguousarray(x[b].T.astype(np.float16)),
            "wq": np.ascontiguousarray(
                W_qkv[:, 0 * C + h0 * D: 0 * C + h0 * D + DCORE].astype(np.float16)),
            "wk": np.ascontiguousarray(
                W_qkv[:, 1 * C + h0 * D: 1 * C + h0 * D + DCORE].astype(np.float16)),
            "wv": np.ascontiguousarray(
                W_qkv[:, 2 * C + h0 * D: 2 * C + h0 * D + DCORE].astype(np.float16)),
            "wp": np.ascontiguousarray(
                W_proj[h0 * D: h0 * D + DCORE, :].astype(np.float16)),
        })
    return in_maps


def kernel(x, W_qkv, W_proj, b_proj, trace=False):
    x = np.asarray(x, dtype=np.float32)
    W_qkv = np.asarray(W_qkv, dtype=np.float32)
    W_proj = np.asarray(W_proj, dtype=np.float32)
    b_proj = np.asarray(b_proj, dtype=np.float32)

    nc = build_nc()
    in_maps = make_in_maps(x, W_qkv, W_proj)

    global LAST_RESULT
    res = run_bass_kernel_spmd(nc, in_maps, list(range(NCORES)), trace=trace)
    LAST_RESULT = res

    out = np.empty((B, N, C), dtype=np.float32)
    for b in range(B):
        out[b] = (res.results[2 * b]["out_partial"]
                  + res.results[2 * b + 1]["out_partial"]
                  + b_proj[None, :])
    return out


# revision 41
# speedup vs baseline: 1.1525x; 1.0557x over previous
"""Multi-head attention (B=4, N=2048, C=1024, H=16) on 8 TRN2 NeuronCores.

Sharding: core = 2*b + half handles batch b, heads half*8 .. half*8+7.
Each core computes QKV for its 8 heads, full attention for them, and a
partial projection (its 512 rows of W_proj). Host sums the two partials
per batch and adds the bias.

All matmul operands are fp16; accumulation stays fp32 in PSUM. The host
pre-casts weights/x to fp16 and pre-transposes x so x^T tiles DMA in
contiguously.

On-chip layout is "transposed": Q^T/K^T [d, n] come straight out of the
QKV matmuls, scores are computed as S^T[m, n] so that exp(S^T) = P^T is
directly the moving operand of the AV matmul (V chunk stationary). exp
is shifted by a constant bias (cancels in softmax) to keep P in fp16
range. Row sums of P ride along as a 65th stationary column of ones.

The whole attention phase is one global stream of 512 "units" (pair,
n-block, m-tile, head), each a [128, 512] S^T score block. Units are
grouped into exp windows that alternate between a 3-bank and a 2-bank
PSUM tile, so the scalar engine's exp stream is fully double-buffered
(while exp reads window k, the PE writes scores into window k+1). AV
matmuls are software-pipelined one window late so the in-order PE
stream never head-of-line blocks on an exp result. Everything else
(QKV for later pairs, V, the projection, softmax normalization's
broadcast+multiply) is drained from a side queue, one unit per window,
into the PE's idle time.

The softmax reciprocal runs on the DVE but only after a DMA packs the
1024 denominators from one partition row across all 128 partitions
(the DVE's iterative divide is ~8 cycles per element per lane); the
result is DMA'd back and broadcast across partitions with two K=1
matmuls, column-tiled into one PSUM bank, so a single tensor_tensor
multiply normalizes both heads of a pair.
"""

import functools
from collections import deque
from contextlib import ExitStack

import numpy as np

import concourse.bass as bass
import concourse.tile as tile
from concourse import bacc, mybir
from concourse.bass_utils import run_bass_kernel_spmd

F32 = mybir.dt.float32
F16 = mybir.dt.float16
AF = mybir.ActivationFunctionType

B, N, C = 4, 2048, 1024
H, D = 16, 64
P = 128
NCORES = 8
HPC = 8            # heads per core
PAIRS = HPC // 2   # 4
DCORE = HPC * D    # 512 attention columns per core
SCALE = float(H) ** -0.5  # 0.25 (faithful to reference: num_heads**-0.5)
EXP_BIAS = -5.0    # exp(scale*s + bias): cancels in softmax, keeps fp16 range
NB = N // 512      # 4 n blocks
NT = N // P        # 16 m tiles of 128
CT = C // P        # 8 contraction chunks
VW = D + 1         # V columns per head incl. the ones column (row sums)
MBLK = HPC * VW    # 520 v_sb columns per m-tile

LAST_RESULT = None  # BassKernelResults of the most recent run (for test.py)


def _kernel_body(tc, out_d, xt_d, wq_d, wk_d, wv_d, wp_d):
    nc = tc.nc
    with ExitStack() as ctx:
        const = ctx.enter_context(tc.tile_pool(name="const", bufs=1))
        ones_f = const.tile([P, P], F32)
        nc.vector.memset(ones_f, 1.0)
        ones_bc = const.tile([P, 64], F16)
        nc.vector.tensor_copy(ones_bc, ones_f[:, 0:64])
        ebias = const.tile([P, 1], F32)
        nc.vector.memset(ebias, EXP_BIAS)

        # attT: pair p occupies cols [p*N, (p+1)*N); partitions = 2 heads x 64
        attT_pool = ctx.enter_context(tc.tile_pool(name="attT", bufs=1))
        attT = attT_pool.tile([P, PAIRS * N], F16)

        # QK weights first: their DMA submissions must not queue behind
        # the 32 x^T pieces on the Sync engine (the first K^T matmul
        # needs wk almost immediately). Pair p at cols [p*C, (p+1)*C),
        # chunk cc within that at [cc*P, (cc+1)*P).
        wqk_pool = ctx.enter_context(tc.tile_pool(name="wqk", bufs=1))
        wq_sb = wqk_pool.tile([P, PAIRS * C], F16)
        wk_sb = wqk_pool.tile([P, PAIRS * C], F16)
        for pq in range(PAIRS):
            for w_d, w_sb in ((wq_d, wq_sb), (wk_d, wk_sb)):
                nc.sync.dma_start(
                    out=w_sb[:, pq * C:(pq + 1) * C].rearrange(
                        "q (cc f) -> q cc f", cc=CT),
                    in_=w_d[:, pq * P:(pq + 1) * P].rearrange(
                        "(cc q) f -> q cc f", q=P))

        # x^T: c-chunk j at cols [j*N, (j+1)*N). DMA'd in (chunk, n-block)
        # pieces so the first K^T block can start ~3us in.
        xt_pool = ctx.enter_context(tc.tile_pool(name="xt", bufs=1))
        xt = xt_pool.tile([P, CT * N], F16)
        for nbb in range(NB):
            for j in range(CT):
                nc.sync.dma_start(
                    out=xt[:, j * N + nbb * 512: j * N + nbb * 512 + 512],
                    in_=xt_d[j * P:(j + 1) * P, nbb * 512: nbb * 512 + 512])

        # V: m-tile m at cols [m*MBLK, ...); head hl at [m*MBLK + hl*VW, +D],
        # then a ones column (for row sums)
        v_pool = ctx.enter_context(tc.tile_pool(name="v", bufs=1))
        v_sb = v_pool.tile([P, NT * MBLK], F16)
        ones_cols = v_sb.rearrange("q (g k) -> q g k", k=VW)[:, :, D:VW]
        nc.vector.tensor_copy(
            ones_cols, ones_f.rearrange("q (g k) -> q g k", k=1))

        wv_pool = ctx.enter_context(tc.tile_pool(name="wv", bufs=1))
        wv_sb = wv_pool.tile([P, CT * DCORE], F16)
        for cc in range(CT):
            nc.sync.dma_start(out=wv_sb[:, cc * DCORE:(cc + 1) * DCORE],
                              in_=wv_d[cc * P:(cc + 1) * P, :])

        wp_pool = ctx.enter_context(tc.tile_pool(name="wp", bufs=1))
        wp_sb = wp_pool.tile([P, PAIRS * C], F16)
        for dc in range(PAIRS):
            nc.sync.dma_start(out=wp_sb[:, dc * C:(dc + 1) * C],
                              in_=wp_d[dc * P:(dc + 1) * P, :])

        # Q^T/K^T for ALL pairs stay resident (4KB/partition each)
        qk_pool = ctx.enter_context(tc.tile_pool(name="qk", bufs=1))
        qts = [qk_pool.tile([P, N], F16, name=f"qt{i}", tag=f"qt{i}")
               for i in range(PAIRS)]
        kts = [qk_pool.tile([P, N], F16, name=f"kt{i}", tag=f"kt{i}")
               for i in range(PAIRS)]

        pt_pool = ctx.enter_context(tc.tile_pool(name="pt", bufs=3))
        den_pool = ctx.enter_context(tc.tile_pool(name="den", bufs=2))
        denp_pool = ctx.enter_context(tc.tile_pool(name="denp", bufs=2))
        rp_pool = ctx.enter_context(tc.tile_pool(name="rp", bufs=2))
        r_pool = ctx.enter_context(tc.tile_pool(name="r", bufs=2))
        tmb_pool = ctx.enter_context(tc.tile_pool(name="tmb", bufs=3))
        stage_pool = ctx.enter_context(tc.tile_pool(name="stage", bufs=3))

        # PSUM: sA 3 + sB 2 + avA/avB 2 + mm 1 = 8 banks
        ps_s = ctx.enter_context(tc.tile_pool(name="ps_s", bufs=1, space="PSUM"))
        ps_av = ctx.enter_context(tc.tile_pool(name="ps_av", bufs=1, space="PSUM"))
        ps_mm = ctx.enter_context(tc.tile_pool(name="ps_mm", bufs=1, space="PSUM"))

        def emit_qk_block(w_sb, pq, nbb, dst):
            """One n-block of Q^T or K^T for pair pq into dst[:, nbb*512...]."""
            psq = ps_mm.tile([P, 512], F32, tag="mm")
            for cc in range(CT):
                nc.tensor.matmul(
                    psq,
                    w_sb[:, pq * C + cc * P: pq * C + (cc + 1) * P],
                    xt[:, cc * N + nbb * 512: cc * N + nbb * 512 + 512],
                    start=(cc == 0), stop=(cc == CT - 1))
            nc.vector.tensor_copy(dst[:, nbb * 512:(nbb + 1) * 512], psq)

        def emit_v_chunk(pq, m):
            """V for head pair pq, m-tile m (2 heads x 64 dims)."""
            psv = ps_mm.tile([P, P], F32, tag="mm")
            for cc in range(CT):
                nc.tensor.matmul(
                    psv,
                    xt[:, cc * N + m * P: cc * N + (m + 1) * P],
                    wv_sb[:, cc * DCORE + pq * P: cc * DCORE + (pq + 1) * P],
                    start=(cc == 0), stop=(cc == CT - 1))
            base = m * MBLK + 2 * pq * VW
            nc.vector.tensor_copy(
                v_sb[:, base: base + 2 * VW].rearrange(
                    "q (h k) -> q h k", k=VW)[:, :, 0:D],
                psv.rearrange("q (h k) -> q h k", k=D))

        def emit_proj_unit(i, co, tag="mm"):
            """One [128, 512] block of this core's partial projection."""
            pool = ps_mm if tag == "mm" else ps_s
            psp = pool.tile([P, 512], F32, name="psp", tag=tag)
            for dc in range(PAIRS):
                nc.tensor.matmul(
                    psp,
                    attT[:, dc * N + i * P: dc * N + (i + 1) * P],
                    wp_sb[:, dc * C + co * 512: dc * C + co * 512 + 512],
                    start=(dc == 0), stop=(dc == PAIRS - 1))
            st = stage_pool.tile([P, 512], F32, tag="st")
            nc.vector.tensor_copy(st, psp)
            nc.sync.dma_start(
                out=out_d[i * P:(i + 1) * P, co * 512: co * 512 + 512],
                in_=st)

        side = deque()

        def emit_norm_front(p, nb, ava, avb):
            """DVE/DMA part of softmax normalization for (pair, nb): raw
            evictions, denominator extraction + packed reciprocal. No PE
            ops (those would head-of-line block the PE stream on the
            multi-us recip chain) — the broadcast+mul go on the side
            queue via emit_norm_back."""
            osl = slice(p * N + nb * 512, p * N + nb * 512 + 512)
            den = den_pool.tile([P, 1024], F32, tag="den")
            nc.vector.tensor_copy(den[64:65, 0:512], ava[D:VW, :])
            nc.vector.tensor_copy(den[64:65, 512:1024], avb[D:VW, :])
            nc.vector.tensor_copy(attT[0:64, osl], ava[0:64, :])
            tmb = tmb_pool.tile([64, 512], F16, tag="tmb")
            nc.vector.tensor_copy(tmb, avb[0:64, :])
            # head B's rows sit at partitions 0-63; shift to 64-127
            nc.sync.dma_start(out=attT[64:128, osl], in_=tmb)
            # r = 1/den: spread the 1024 denominators across all 128
            # partitions (DVE recip is ~8 cyc/elem PER LANE), recip, unpack
            denP = denp_pool.tile([P, 8], F32, tag="denp")
            nc.sync.dma_start(out=denP, in_=den[64:65, :])
            rP = rp_pool.tile([P, 8], F16, tag="rp")
            with nc.allow_low_precision(
                    reason="softmax recip rounding is benign"):
                nc.vector.reciprocal(rP, denP)
            r = r_pool.tile([P, 1024], F16, tag="r")
            nc.sync.dma_start(out=r[64:65, :], in_=rP)
            return r

        def emit_norm_back(p, nb, r):
            """PE+DVE tail of normalization: broadcast r across partitions
            (two K=1 matmuls col-tiled into one bank) and multiply."""
            osl = slice(p * N + nb * 512, p * N + nb * 512 + 512)
            ps_bc = ps_mm.tile([P, 512], F32, tag="mm")
            nc.tensor.matmul(
                ps_bc[0:64, :], ones_bc[64:65, :], r[64:65, 0:512],
                start=True, stop=True, tile_position=(64, 0),
                skip_group_check=True)
            nc.tensor.matmul(
                ps_bc[64:128, :], ones_bc[64:65, :], r[64:65, 512:1024],
                start=True, stop=True, tile_position=(64, 64),
                skip_group_check=True)
            with nc.allow_low_precision(
                    reason="softmax normalization rounding is benign"):
                nc.vector.tensor_mul(attT[:, osl], attT[:, osl], ps_bc)
            if p == PAIRS - 1:
                for k, i in enumerate(range(4 * nb, 4 * nb + 4)):
                    for co in range(2):
                        # the last n-block's projection runs at the kernel
                        # tail where the score banks are free: alternate
                        # PSUM pools so its 8 units double-buffer instead
                        # of serializing through the single mm bank. These
                        # units depend on the very last normalization, so
                        # the change cannot perturb earlier scheduling.
                        tag = ("sB" if (nb == NB - 1 and (k * 2 + co) % 2)
                               else "mm")
                        side.append(
                            functools.partial(emit_proj_unit, i, co, tag))

        # ---- startup: K^T fully and Q^T n-block 0 for pair 0 ----
        for nbb in range(NB):
            emit_qk_block(wk_sb, 0, nbb, kts[0])
        emit_qk_block(wq_sb, 0, 0, qts[0])
        for nbb in range(1, NB):
            side.append(functools.partial(emit_qk_block, wq_sb, 0, nbb, qts[0]))

        # ---- the global attention stream ----
        units = [(p, nb, m, h)
                 for p in range(PAIRS) for nb in range(NB)
                 for m in range(NT) for h in range(2)]
        pending = []        # last window's (unit, pt, slot) awaiting AV
        av_cur = {}         # (p, nb) -> (avA, avB) accumulation tiles
        v0_done = set()     # pair-0 m-tiles whose V has been emitted
        started = set()     # pairs whose side work has been queued

        def emit_av(u, pt, slot):
            p, nb, m, h = u
            if (p, nb) not in av_cur:
                av_cur[(p, nb)] = (
                    ps_av.tile([P, 512], F32, name="ava", tag="avA"),
                    ps_av.tile([P, 512], F32, name="avb", tag="avB"))
            ava, avb = av_cur[(p, nb)]
            vbase = m * MBLK + (2 * p + h) * VW
            nc.tensor.matmul(
                (ava if h == 0 else avb)[0:VW, :],
                v_sb[:, vbase: vbase + VW],
                pt[:, slot * 512:(slot + 1) * 512],
                start=(m == 0), stop=(m == NT - 1),
                skip_group_check=True)
            if m == NT - 1 and h == 1:
                r = emit_norm_front(p, nb, ava, avb)
                side.append(functools.partial(emit_norm_back, p, nb, r))
                del av_cur[(p, nb)]

        ui = 0
        toggle = 0
        wi = 0
        while ui < len(units):
            wsize = 3 if toggle == 0 else 2
            wunits = units[ui:ui + wsize]
            for (p, nb, m, h) in wunits:
                if p not in started:
                    started.add(p)
                    if p < PAIRS - 1:
                        for nbb in range(NB):
                            side.append(functools.partial(
                                emit_qk_block, wk_sb, p + 1, nbb, kts[p + 1]))
                        for nbb in range(NB):
                            side.append(functools.partial(
                                emit_qk_block, wq_sb, p + 1, nbb, qts[p + 1]))
                        for m2 in range(NT):
                            side.append(functools.partial(
                                emit_v_chunk, p + 1, m2))
                if p == 0 and nb == 0 and m not in v0_done:
                    v0_done.add(m)
                    emit_v_chunk(0, m)
            pss = ps_s.tile([P, 512 * wsize], F32,
                            tag=("sA" if toggle == 0 else "sB"))
            for i, (p, nb, m, h) in enumerate(wunits):
                nsl = slice(nb * 512, nb * 512 + 512)
                nc.tensor.matmul(
                    pss[:, i * 512:(i + 1) * 512],
                    kts[p][h * 64:(h + 1) * 64, m * P:(m + 1) * P],
                    qts[p][h * 64:(h + 1) * 64, nsl],
                    start=True, stop=True)
            pt = pt_pool.tile([P, 512 * wsize], F16,
                              tag=("ptA" if toggle == 0 else "ptB"))
            nc.scalar.activation(pt, pss, AF.Exp, scale=SCALE, bias=ebias)
            # AV of the PREVIOUS window (software pipeline: keeps ready PE
            # work queued while this window's exp runs)
            for u, ptt, slot in pending:
                emit_av(u, ptt, slot)
            pending = [(u, pt, i) for i, u in enumerate(wunits)]
            in_p0nb0 = wunits[0][0] == 0 and wunits[0][1] == 0
            if side and (wi >= 8 or not in_p0nb0):
                side.popleft()()
            ui += wsize
            toggle ^= 1
            wi += 1
        for u, ptt, slot in pending:
            emit_av(u, ptt, slot)
        while side:
            side.popleft()()


@functools.lru_cache(maxsize=1)
def build_nc():
    nc = bacc.Bacc("TRN2", target_bir_lowering=False, debug=False)
    xt_d = nc.dram_tensor("xt_local", [C, N], F16, kind="ExternalInput").ap()
    wq_d = nc.dram_tensor("wq", [C, DCORE], F16, kind="ExternalInput").ap()
    wk_d = nc.dram_tensor("wk", [C, DCORE], F16, kind="ExternalInput").ap()
    wv_d = nc.dram_tensor("wv", [C, DCORE], F16, kind="ExternalInput").ap()
    wp_d = nc.dram_tensor("wp", [DCORE, C], F16, kind="ExternalInput").ap()
    out_d = nc.dram_tensor("out_partial", [N, C], F32, kind="ExternalOutput").ap()
    with tile.TileContext(nc) as tc:
        _kernel_body(tc, out_d, xt_d, wq_d, wk_d, wv_d, wp_d)
    nc.compile()
    return nc


def make_in_maps(x, W_qkv, W_proj):
    in_maps = []
    for core in range(NCORES):
        b, half = core // 2, core % 2
        h0 = half * HPC
        in_maps.append({
            "xt_local": np.ascontiguousarray(x[b].T.astype(np.float16)),
            "wq": np.ascontiguousarray(
                W_qkv[:, 0 * C + h0 * D: 0 * C + h0 * D + DCORE].astype(np.float16)),
            "wk": np.ascontiguousarray(
                W_qkv[:, 1 * C + h0 * D: 1 * C + h0 * D + DCORE].astype(np.float16)),
            "wv": np.ascontiguousarray(
                W_qkv[:, 2 * C + h0 * D: 2 * C + h0 * D + DCORE].astype(np.float16)),
            "wp": np.ascontiguousarray(
                W_proj[h0 * D: h0 * D + DCORE, :].astype(np.float16)),
        })
    return in_maps


def kernel(x, W_qkv, W_proj, b_proj, trace=False):
    x = np.asarray(x, dtype=np.float32)
    W_qkv = np.asarray(W_qkv, dtype=np.float32)
    W_proj = np.asarray(W_proj, dtype=np.float32)
    b_proj = np.asarray(b_proj, dtype=np.float32)

    nc = build_nc()
    in_maps = make_in_maps(x, W_qkv, W_proj)

    global LAST_RESULT
    res = run_bass_kernel_spmd(nc, in_maps, list(range(NCORES)), trace=trace)
    LAST_RESULT = res

    out = np.empty((B, N, C), dtype=np.float32)
    for b in range(B):
        out[b] = (res.results[2 * b]["out_partial"]
                  + res.results[2 * b + 1]["out_partial"]
                  + b_proj[None, :])
    return out
